# revision 1
# baseline (speedup 1.0000x reference)
"""Trainium2 Bass kernel for CausalPriorityAttention.

Data-parallel over the batch dim: core b computes batch b (B=8, 8 cores).

Per-core dataflow (matmuls in float32r / fp16, N=512 free dim):
  phase 1: qkT = W_qk @ x^T  (Q^T,K^T in [feat, seq] layout, f32r)
           V   = x @ W_v^T   (natural [seq, feat] layout, +ones col, fp16)
           E   = exp(10a * sigmoid(graph_bias))  (fp16, shared across heads)
  phase 2 (per head pair, row-group-paired K=64 score matmuls):
           sT[k,q] = K @ Q'^T            (transposed scores -> PSUM)
           probs   = exp(10a*sT - 5) * E (ACT exp + DVE fp16 2x-mode mult)
           pv[65,q] = [V_h | 1]^T @ probs  (out^T rows + rowsums via ones col)
           attnT = pv[0:64] * recip(pv[64])  (gpsimd partition_broadcast)
  phase 3: y = attnT^T @ Wo^T + bo
Q' is prescaled by 1/(8*10a) so exp's scale=10a restores QK/8; the
reference bias's constant -5a term drops out of softmax, and the -5 shift
(which also cancels in normalization) keeps exp products in fp16 range.
The transposed-score layout means graph_bias is consumed untransposed and
probs come out pre-transposed for the PV matmul: zero on-chip transposes.
"""

import sys

for _p in ("/opt/trn_rl_repo",):
    if _p not in sys.path:
        sys.path.append(_p)

import numpy as np

import concourse.bacc as bacc
import concourse.bass as bass
import concourse.mybir as mybir
import concourse.tile as tile
from concourse.bass_utils import run_bass_kernel_spmd

B, N, D = 8, 1024, 512
H, HD = 8, 64
P = 128
NT = N // P          # 8 seq tiles
KT = D // P          # 4 contraction tiles over D
FT_QK = 2 * D // P   # 8 feature tiles over [Q;K]
QC = N // 512        # 2 q-chunks of 512
F32 = mybir.dt.float32
F32R = mybir.dt.float32r
F16 = mybir.dt.float16

_CACHE = {}


def build_nc(ten_a: float, reps: int = 1):
    import os
    no_bias = bool(os.environ.get("VAR_NO_BIAS"))
    no_norm = bool(os.environ.get("VAR_NO_NORM"))
    dma_bcast = bool(os.environ.get("VAR_DMA_BCAST"))
    nc = bacc.Bacc("TRN2")
    xT = nc.dram_tensor("xT", [D, N], F32R, kind="ExternalInput")
    wT = nc.dram_tensor("wT", [D, 3 * D], F32R, kind="ExternalInput")
    woT = nc.dram_tensor("woT", [D, D], F32R, kind="ExternalInput")
    gb = nc.dram_tensor("gb", [N, N], F32, kind="ExternalInput")
    qkb = nc.dram_tensor("qkb", [P, FT_QK], F32, kind="ExternalInput")
    ones8 = nc.dram_tensor("ones8", [H], F16, kind="ExternalInput")
    vb = nc.dram_tensor("vb", [D], F32, kind="ExternalInput")
    bo = nc.dram_tensor("bo", [1, D], F32R, kind="ExternalInput")
    ones1 = nc.dram_tensor("ones1", [1, P], F32R, kind="ExternalInput")
    y = nc.dram_tensor("y", [N, D], F32, kind="ExternalOutput")

    sQ = 1.0 / (8.0 * ten_a)

    with tile.TileContext(nc) as tc:
        with tc.tile_pool(name="const", bufs=1) as const_pool, \
             tc.tile_pool(name="persist", bufs=1) as persist:
            qkb_sb = const_pool.tile([P, FT_QK], F32)
            nc.sync.dma_start(out=qkb_sb, in_=qkb[:, :])
            vb_sb = const_pool.tile([P, D], F32)
            nc.sync.dma_start(
                out=vb_sb,
                in_=bass.AP(tensor=vb.ap().tensor, offset=0, ap=[[0, P], [1, D]]),
            )
            bo_sb = const_pool.tile([1, D], F32R)
            nc.sync.dma_start(out=bo_sb, in_=bo[:, :])
            ones1_sb = const_pool.tile([1, P], F32R)
            nc.sync.dma_start(out=ones1_sb, in_=ones1[:, :])
            neg5 = const_pool.tile([P, 1], F32)
            nc.vector.memset(neg5, -5.0)

            qkT_sb = persist.tile([P, FT_QK, N], F32R)
            v_sb = persist.tile([P, NT, H, HD + 1], F16)
            for st in range(NT):
                nc.sync.dma_start(
                    out=v_sb[:, st, :, HD : HD + 1],
                    in_=bass.AP(tensor=ones8, offset=0, ap=[[0, P], [1, H]]),
                )
            e_sb = persist.tile([P, NT, N], F16)
            attnT_sb = persist.tile([P, KT, N], F32R)
            woT_sb = persist.tile([P, KT, D], F32R)
            nc.sync.dma_start(
                out=woT_sb, in_=woT[:, :].rearrange("(t p) n -> p t n", p=P)
            )

            for _rep in range(reps):
                # ---- phase 1: projections + sigmoid(graph_bias) ----
                with tc.tile_pool(name="ph1", bufs=1) as ph1, \
                     tc.tile_pool(name="ps1", bufs=8, space="PSUM") as ps1:
                    xT_sb = ph1.tile([P, KT, N], F32R, name="xT_sb")
                    wT_sb = ph1.tile([P, KT, 3 * D], F32R, name="wT_sb")
                    for k in range(KT):
                        nc.sync.dma_start(
                            out=xT_sb[:, k, :], in_=xT[k * P : (k + 1) * P, :]
                        )
                        nc.sync.dma_start(
                            out=wT_sb[:, k, :], in_=wT[k * P : (k + 1) * P, :]
                        )
                    # E = exp(10a * sigmoid(graph_bias)); shared across heads.
                    # All sigmoids batch before all exps (one table switch).
                    sig_sb = ph1.tile([P, NT, N], F32, name="sig_sb")
                    for kt in range(NT):
                        nc.sync.dma_start(
                            out=sig_sb[:, kt, :], in_=gb[kt * P : (kt + 1) * P, :]
                        )
                        nc.scalar.activation(
                            out=sig_sb[:, kt, :],
                            in_=sig_sb[:, kt, :],
                            func=mybir.ActivationFunctionType.Sigmoid,
                        )
                    for kt in range(NT):
                        nc.scalar.activation(
                            out=e_sb[:, kt, :],
                            in_=sig_sb[:, kt, :],
                            func=mybir.ActivationFunctionType.Exp,
                            scale=ten_a,
                        )

                    # Projections, k OUTERMOST within blocks of 4 psum
                    # groups: k=0..2 matmuls run as each x/w DMA chunk lands
                    # instead of all groups waiting for the last chunk.
                    # Group order feeds head-pair 0 (ft 0,4) first.
                    qk_groups = []
                    for ft in (0, 4, 1, 5, 2, 6, 3, 7):
                        for qc in range(QC):
                            qk_groups.append(("qk", ft, qc))
                    for st in range(NT):
                        qk_groups.append(("v", st, 0))

                    for b0 in range(0, len(qk_groups), 4):
                        blk = qk_groups[b0 : b0 + 4]
                        tiles = [
                            ps1.tile([P, 512], F32, tag="ps1", name="ps")
                            for _ in blk
                        ]
                        for k in range(KT):
                            for g, t in zip(blk, tiles):
                                if g[0] == "qk":
                                    _, ft, qc = g
                                    nc.tensor.matmul(
                                        t,
                                        lhsT=wT_sb[:, k, ft * P : (ft + 1) * P],
                                        rhs=xT_sb[:, k, qc * 512 : (qc + 1) * 512],
                                        start=(k == 0),
                                        stop=(k == KT - 1),
                                    )
                                else:
                                    _, st, _ = g
                                    nc.tensor.matmul(
                                        t,
                                        lhsT=xT_sb[:, k, st * P : (st + 1) * P],
                                        rhs=wT_sb[:, k, 2 * D : 3 * D],
                                        start=(k == 0),
                                        stop=(k == KT - 1),
                                    )
                        for g, t in zip(blk, tiles):
                            if g[0] == "qk":
                                _, ft, qc = g
                                # copy out with per-partition bias + Q prescale
                                nc.vector.tensor_scalar(
                                    out=qkT_sb[:, ft, qc * 512 : (qc + 1) * 512],
                                    in0=t,
                                    scalar1=qkb_sb[:, ft : ft + 1],
                                    scalar2=(sQ if ft < FT_QK // 2 else 1.0),
                                    op0=mybir.AluOpType.add,
                                    op1=mybir.AluOpType.mult,
                                )
                            else:
                                _, st, _ = g
                                nc.vector.tensor_tensor(
                                    out=v_sb[:, st, :, 0:HD],
                                    in0=t.rearrange("p (h d) -> p h d", h=H),
                                    in1=vb_sb.rearrange("p (h d) -> p h d", h=H),
                                    op=mybir.AluOpType.add,
                                )

                # ---- phase 2: attention per head-pair ----
                with tc.tile_pool(name="ph2", bufs=1) as ph2, \
                     tc.tile_pool(name="small", bufs=8) as small, \
                     tc.tile_pool(name="ps_s", bufs=3, space="PSUM") as ps_s, \
                     tc.tile_pool(name="ps_pv", bufs=2, space="PSUM") as ps_pv:
                    for hp in range(H // 2):
                        # expT double-buffered (bufs=2): pair hp's exps don't
                        # wait for pair hp-1's PV reads of the same buffer
                        expT = [
                            ph2.tile([P, NT, N], F16, tag="exp0", bufs=2,
                                     name="expT0"),
                            ph2.tile([P, NT, N], F16, tag="exp1", bufs=2,
                                     name="expT1"),
                        ]
                        # scores for both heads of the pair; adjacent MMs land
                        # on different PE row groups and run concurrently
                        for kt in range(NT):
                            sT2 = [
                                ps_s.tile([P, N], F32, tag="sT", name="sTa"),
                                ps_s.tile([P, N], F32, tag="sT", name="sTb"),
                            ]
                            for qc in range(QC):
                                for sub in range(2):
                                    qp = 64 * sub
                                    nc.tensor.matmul(
                                        sT2[sub][:, qc * 512 : (qc + 1) * 512],
                                        lhsT=qkT_sb[
                                            qp : qp + HD,
                                            FT_QK // 2 + hp,
                                            kt * P : (kt + 1) * P,
                                        ],
                                        rhs=qkT_sb[
                                            qp : qp + HD, hp, qc * 512 : (qc + 1) * 512
                                        ],
                                        start=True,
                                        stop=True,
                                    )
                            for sub in range(2):
                                # -5 keeps exp(s)+exp(bias) products in fp16
                                # range; the shift cancels in normalization
                                es = ph2.tile([P, N], F16, tag="es", bufs=4,
                                              name="es")
                                nc.scalar.activation(
                                    out=es,
                                    in_=sT2[sub],
                                    func=mybir.ActivationFunctionType.Exp,
                                    scale=ten_a,
                                    bias=neg5,
                                )
                                nc.vector.tensor_tensor(
                                    out=expT[sub][:, kt, :],
                                    in0=es,
                                    in1=e_sb[:, kt, :],
                                    op=mybir.AluOpType.mult,
                                )
                        for sub in range(2):
                            h = 2 * hp + sub
                            qp = 64 * sub
                            for qc in range(QC):
                                pv = ps_pv.tile([HD + 1, 512], F32, tag="pv", name="pv")
                                for kt in range(NT):
                                    nc.tensor.matmul(
                                        pv,
                                        lhsT=v_sb[:, kt, h, :],
                                        rhs=expT[sub][:, kt, qc * 512 : (qc + 1) * 512],
                                        start=(kt == 0),
                                        stop=(kt == NT - 1),
                                    )
                                attn_out = attnT_sb[
                                    qp : qp + HD, hp, qc * 512 : (qc + 1) * 512
                                ]
                                if no_norm:
                                    nc.vector.tensor_scalar(
                                        out=attn_out, in0=pv[0:HD, :], scalar1=1.0,
                                        scalar2=None, op0=mybir.AluOpType.mult,
                                    )
                                else:
                                    recip = small.tile([1, 512], F32, tag="recip", name="recip")
                                    nc.vector.reciprocal(recip, pv[HD : HD + 1, :])
                                    bc = small.tile([HD, 512], F32, tag="bc", name="bc")
                                    if dma_bcast:
                                        nc.sync.dma_start(
                                            out=bc,
                                            in_=bass.AP(
                                                tensor=recip.tensor,
                                                offset=recip.offset,
                                                ap=[[0, HD]] + list(recip.ap[1:]),
                                            ),
                                        )
                                    else:
                                        nc.gpsimd.partition_broadcast(bc, recip)
                                    nc.vector.tensor_tensor(
                                        out=attn_out,
                                        in0=pv[0:HD, :],
                                        in1=bc,
                                        op=mybir.AluOpType.mult,
                                    )

                # ---- phase 3: output projection ----
                with tc.tile_pool(name="ph3", bufs=2) as ph3, \
                     tc.tile_pool(name="ps_y", bufs=2, space="PSUM") as ps_y:
                    for st in range(NT):
                        yp = ps_y.tile([P, D], F32, tag="yp", name="yp")
                        for ft in range(KT):
                            nc.tensor.matmul(
                                yp,
                                lhsT=attnT_sb[:, ft, st * P : (st + 1) * P],
                                rhs=woT_sb[:, ft, :],
                                start=(ft == 0),
                                stop=False,
                            )
                        # rank-1 ones-row matmul adds bo into the psum
                        nc.tensor.matmul(
                            yp, lhsT=ones1_sb, rhs=bo_sb, start=False, stop=True
                        )
                        ysb = ph3.tile([P, D], F32, tag="ysb", name="ysb")
                        nc.scalar.copy(out=ysb, in_=yp)
                        nc.sync.dma_start(
                            out=y[st * P : (st + 1) * P, :], in_=ysb
                        )
    nc.finalize()
    return nc


def kernel(x, graph_bias, in_proj_w, in_proj_b, out_proj_w, out_proj_b,
           bias_strength):
    x = np.asarray(x, dtype=np.float32)
    graph_bias = np.ascontiguousarray(np.asarray(graph_bias, dtype=np.float32))
    in_proj_w = np.asarray(in_proj_w, dtype=np.float32)
    in_proj_b = np.asarray(in_proj_b, dtype=np.float32)
    out_proj_w = np.asarray(out_proj_w, dtype=np.float32)
    out_proj_b = np.asarray(out_proj_b, dtype=np.float32)
    alpha = 1.0 / (1.0 + np.exp(-float(np.asarray(bias_strength))))
    ten_a = 10.0 * alpha

    key = round(ten_a, 9)
    if key not in _CACHE:
        _CACHE[key] = build_nc(ten_a)
    nc = _CACHE[key]

    wT = np.ascontiguousarray(in_proj_w.T)          # [512, 1536]
    woT = np.ascontiguousarray(out_proj_w.T)        # [512, 512]
    qkb = np.ascontiguousarray(
        in_proj_b[: 2 * D].reshape(FT_QK, P).T      # [128, 8]
    )
    vb = in_proj_b[2 * D :]
    bo = out_proj_b

    in_maps = []
    for b in range(B):
        in_maps.append({
            "xT": np.ascontiguousarray(x[b].T),
            "wT": wT,
            "woT": woT,
            "gb": graph_bias[b],
            "qkb": qkb,
            "ones8": np.ones(H, dtype=np.float16),
            "vb": vb,
            "bo": bo.reshape(1, D),
            "ones1": np.ones((1, P), dtype=np.float32),
        })

    global _saved_in_maps
    _saved_in_maps = in_maps
    res = run_bass_kernel_spmd(nc, in_maps, core_ids=list(range(B)))
    out = np.stack([res.results[b]["y"] for b in range(B)], axis=0)
    return out.astype(np.float32)



# revision 8
# speedup vs baseline: 1.3271x; 1.3271x over previous
"""Trainium2 Bass kernel for CausalPriorityAttention.

Data-parallel over the batch dim: core b computes batch b (B=8, 8 cores).

v3: fp16 dataflow end-to-end, DMA dispatch spread across queues
(SP/Pool), x^T and W^T host-packed into one dram tensor, and a fused
phase-1/phase-2 emission schedule tuned for the in-order engine queues:
pair-0's score matmuls are emitted right after pair-0's QKV groups (so
the ACT engine flows from sigmoid/E straight into score exps with no
gap), pair p+1's scores are emitted before pair p's PV (lookahead keeps
ACT saturated), and pair-0's probs multiplies are deferred until after
all phase-1 DVE copy-outs (in-order DVE would otherwise stall phase 1).

Per-core dataflow (512-wide matmuls, fp16 operands, f32 psum):
  phase 1: qkT = W_qk @ x^T  (Q^T,K^T in [feat, seq] layout)
           V   = x @ W_v^T   (natural [seq, feat] layout, +ones col)
           E   = exp(10a * sigmoid(graph_bias))  (shared across heads)
  phase 2 (per head pair, row-group-paired K=64 score matmuls):
           sT[k,q] = K @ Q'^T            (transposed scores -> PSUM)
           probs   = exp(10a*sT - 5) * E (ACT exp + DVE fp16 mult)
           pv[65,q] = [V_h | 1]^T @ probs  (out^T rows + rowsums)
           attnT = pv[0:64] * recip(pv[64])  (partition_broadcast)
  phase 3: y = attnT^T @ Wo^T + bo, emitted per q-chunk as soon as the
           last head pair's normalization for it lands
Q' is prescaled by 1/(8*10a) so exp's scale=10a restores QK/8; the
reference bias's constant -5a term drops out of softmax, and the -5 shift
(which also cancels in normalization) keeps exp products in fp16 range.
The transposed-score layout means graph_bias is consumed untransposed and
probs come out pre-transposed for the PV matmul: zero on-chip transposes.
"""

import sys

for _p in ("/opt/trn_rl_repo",):
    if _p not in sys.path:
        sys.path.append(_p)

import numpy as np

import concourse.bacc as bacc
import concourse.bass as bass
import concourse.mybir as mybir
import concourse.tile as tile
from concourse.bass_utils import run_bass_kernel_spmd

B, N, D = 8, 1024, 512
H, HD = 8, 64
P = 128
NT = N // P          # 8 seq tiles
KT = D // P          # 4 contraction tiles over D
FT_QK = 2 * D // P   # 8 feature tiles over [Q;K]
QC = N // 512        # 2 q-chunks of 512
XW = N + 3 * D       # 2560 cols of host-packed [x^T | W^T]
F32 = mybir.dt.float32
F16 = mybir.dt.float16

_CACHE = {}


def build_nc(ten_a: float, reps: int = 1):
    nc = bacc.Bacc("TRN2")
    xw = nc.dram_tensor("xw", [D, XW], F16, kind="ExternalInput")
    gb = nc.dram_tensor("gb", [N, N], F16, kind="ExternalInput")
    woT = nc.dram_tensor("woT", [D, D], F16, kind="ExternalInput")
    qkb = nc.dram_tensor("qkb", [P, FT_QK], F32, kind="ExternalInput")
    vb = nc.dram_tensor("vb", [D], F16, kind="ExternalInput")
    bo = nc.dram_tensor("bo", [1, D], F16, kind="ExternalInput")
    ones1 = nc.dram_tensor("ones1", [1, P], F16, kind="ExternalInput")
    y = nc.dram_tensor("y", [N, D], F32, kind="ExternalOutput")

    sQ = 1.0 / (8.0 * ten_a)

    with tile.TileContext(nc) as tc:
        with tc.tile_pool(name="const", bufs=1) as const_pool, \
             tc.tile_pool(name="persist", bufs=1) as persist:
            qkb_sb = const_pool.tile([P, FT_QK], F32)
            nc.sync.dma_start(out=qkb_sb, in_=qkb[:, :])
            vb_sb = const_pool.tile([P, D], F16)
            nc.sync.dma_start(
                out=vb_sb,
                in_=bass.AP(tensor=vb.ap().tensor, offset=0, ap=[[0, P], [1, D]]),
            )
            bo_sb = const_pool.tile([1, D], F16)
            nc.sync.dma_start(out=bo_sb, in_=bo[:, :])
            ones1_sb = const_pool.tile([1, P], F16)
            nc.sync.dma_start(out=ones1_sb, in_=ones1[:, :])
            neg5 = const_pool.tile([P, 1], F32)
            nc.vector.memset(neg5, -5.0)
            # E = exp(10a*sigmoid(x)) = exp(5a*tanh(x/2) + 5a); Tanh lives in
            # the same ACT table set as Exp, so no table reloads anywhere.
            p5a = const_pool.tile([P, 1], F32)
            nc.vector.memset(p5a, ten_a / 2.0)

            qkT_sb = persist.tile([P, FT_QK, N], F16)
            v_sb = persist.tile([P, NT, H, HD + 1], F16)
            nc.gpsimd.memset(v_sb[:, :, :, HD : HD + 1], 1.0)
            e_sb = persist.tile([P, NT, N], F16)
            attnT_sb = persist.tile([P, KT, N], F16)
            woT_sb = persist.tile([P, KT, D], F16)
            nc.gpsimd.dma_start(
                out=woT_sb, in_=woT[:, :].rearrange("(t p) n -> p t n", p=P)
            )

            for _rep in range(reps):
                with tc.tile_pool(name="ph2", bufs=1) as ph2, \
                     tc.tile_pool(name="small", bufs=2) as small, \
                     tc.tile_pool(name="ps_s", bufs=3, space="PSUM") as ps_s:

                    # -------- emission helpers (shared across phases) -----
                    def emit_scores_exps(hp):
                        """Score matmuls + ACT exps for head pair hp.
                        Returns the es tiles; DVE multiplies are emitted
                        separately (emit_mults) to control DVE queue order."""
                        es_tiles = []
                        for kt in range(NT):
                            sT2 = [
                                ps_s.tile([P, N], F32, tag="sT", name="sTa"),
                                ps_s.tile([P, N], F32, tag="sT", name="sTb"),
                            ]
                            for qc in range(QC):
                                for sub in range(2):
                                    qp = 64 * sub
                                    nc.tensor.matmul(
                                        sT2[sub][:, qc * 512 : (qc + 1) * 512],
                                        lhsT=qkT_sb[
                                            qp : qp + HD,
                                            FT_QK // 2 + hp,
                                            kt * P : (kt + 1) * P,
                                        ],
                                        rhs=qkT_sb[
                                            qp : qp + HD, hp,
                                            qc * 512 : (qc + 1) * 512,
                                        ],
                                        start=True,
                                        stop=True,
                                    )
                            for sub in range(2):
                                # -5 keeps exp(s)*exp(bias) products in fp16
                                # range; the shift cancels in normalization
                                es = ph2.tile([P, N], F16, tag="es", bufs=8,
                                              name="es")
                                nc.scalar.activation(
                                    out=es,
                                    in_=sT2[sub],
                                    func=mybir.ActivationFunctionType.Exp,
                                    scale=ten_a,
                                    bias=neg5,
                                )
                                es_tiles.append(es)
                        return es_tiles

                    def new_expT():
                        return [
                            ph2.tile([P, NT, N], F16, tag="exp0", bufs=2,
                                     name="expT0"),
                            ph2.tile([P, NT, N], F16, tag="exp1", bufs=2,
                                     name="expT1"),
                        ]

                    def emit_mults(es_tiles, expT):
                        for kt in range(NT):
                            for sub in range(2):
                                nc.vector.tensor_tensor(
                                    out=expT[sub][:, kt, :],
                                    in0=es_tiles[2 * kt + sub],
                                    in1=e_sb[:, kt, :],
                                    op=mybir.AluOpType.mult,
                                )

                    def emit_pv_norm(hp, expT, qc, ps_acc):
                        for sub in range(2):
                            h = 2 * hp + sub
                            qp = 64 * sub
                            acc = ps_acc.tile([P, 512], F32, tag="acc",
                                              name="acc")
                            pv = acc[0 : HD + 1, :]
                            for kt in range(NT):
                                nc.tensor.matmul(
                                    pv,
                                    lhsT=v_sb[:, kt, h, :],
                                    rhs=expT[sub][:, kt, qc * 512 : (qc + 1) * 512],
                                    start=(kt == 0),
                                    stop=(kt == NT - 1),
                                )
                            attn_out = attnT_sb[
                                qp : qp + HD, hp, qc * 512 : (qc + 1) * 512
                            ]
                            recip = small.tile([1, 512], F32, tag="recip",
                                               name="recip")
                            nc.vector.reciprocal(recip, pv[HD : HD + 1, :])
                            bc = small.tile([HD, 512], F32, tag="bc", name="bc")
                            nc.gpsimd.partition_broadcast(bc, recip)
                            nc.vector.tensor_tensor(
                                out=attn_out,
                                in0=pv[0:HD, :],
                                in1=bc,
                                op=mybir.AluOpType.mult,
                            )

                    def emit_ph3(qc, ps_acc):
                        for st in range(qc * NT // 2, (qc + 1) * NT // 2):
                            yp = ps_acc.tile([P, D], F32, tag="acc", name="yp")
                            for ft in range(KT):
                                nc.tensor.matmul(
                                    yp,
                                    lhsT=attnT_sb[:, ft, st * P : (st + 1) * P],
                                    rhs=woT_sb[:, ft, :],
                                    start=(ft == 0),
                                    stop=False,
                                )
                            # rank-1 ones-row matmul adds bo into the psum
                            nc.tensor.matmul(
                                yp, lhsT=ones1_sb, rhs=bo_sb,
                                start=False, stop=True,
                            )
                            ysb = ph2.tile([P, D], F32, tag="ysb", bufs=2,
                                           name="ysb")
                            nc.vector.tensor_scalar(
                                out=ysb, in0=yp, scalar1=1.0, scalar2=None,
                                op0=mybir.AluOpType.mult,
                            )
                            nc.sync.dma_start(
                                out=y[st * P : (st + 1) * P, :], in_=ysb
                            )

                    # ---- phase 1 (+ early pair-0 scores/exps) ----
                    with tc.tile_pool(name="ph1", bufs=1) as ph1, \
                         tc.tile_pool(name="ps1", bufs=2, space="PSUM") as ps1:
                        xw_sb = ph1.tile([P, KT, XW], F16, name="xw_sb")
                        for k in range(KT):
                            nc.sync.dma_start(
                                out=xw_sb[:, k, :], in_=xw[k * P : (k + 1) * P, :]
                            )
                        gbt = ph1.tile([P, NT, N], F16, name="gbt")
                        sig = ph1.tile([P, NT, N], F16, name="sig")
                        for kt in range(NT):
                            nc.gpsimd.dma_start(
                                out=gbt[:, kt, :], in_=gb[kt * P : (kt + 1) * P, :]
                            )
                        # E = exp(5a*tanh(x/2) + 5a) = exp(10a*sigmoid(x));
                        # 2-kt tiles amortize the ACT access bubble.
                        for k2 in range(NT // 2):
                            nc.scalar.activation(
                                out=sig[:, 2 * k2 : 2 * k2 + 2, :],
                                in_=gbt[:, 2 * k2 : 2 * k2 + 2, :],
                                func=mybir.ActivationFunctionType.Tanh,
                                scale=0.5,
                            )
                        for k2 in range(NT // 2):
                            nc.scalar.activation(
                                out=e_sb[:, 2 * k2 : 2 * k2 + 2, :],
                                in_=sig[:, 2 * k2 : 2 * k2 + 2, :],
                                func=mybir.ActivationFunctionType.Exp,
                                scale=ten_a / 2.0,
                                bias=p5a,
                            )

                        # Projection groups; k OUTERMOST within blocks of 2
                        # psum groups so k=0..2 matmuls run as each xw DMA
                        # chunk lands. Pair-0's Q/K groups come first, then
                        # its scores+exps, then the rest of phase 1.
                        def emit_blocks(groups):
                            for b0 in range(0, len(groups), 2):
                                blk = groups[b0 : b0 + 2]
                                tiles = [
                                    ps1.tile([P, 512], F32, tag="ps1", name="ps")
                                    for _ in blk
                                ]
                                for k in range(KT):
                                    for g, t in zip(blk, tiles):
                                        if g[0] == "qk":
                                            _, ft, qc = g
                                            nc.tensor.matmul(
                                                t,
                                                lhsT=xw_sb[
                                                    :, k,
                                                    N + ft * P : N + (ft + 1) * P,
                                                ],
                                                rhs=xw_sb[
                                                    :, k, qc * 512 : (qc + 1) * 512
                                                ],
                                                start=(k == 0),
                                                stop=(k == KT - 1),
                                            )
                                        else:
                                            _, st, _ = g
                                            nc.tensor.matmul(
                                                t,
                                                lhsT=xw_sb[
                                                    :, k, st * P : (st + 1) * P
                                                ],
                                                rhs=xw_sb[
                                                    :, k, N + 2 * D : N + 3 * D
                                                ],
                                                start=(k == 0),
                                                stop=(k == KT - 1),
                                            )
                                for g, t in zip(blk, tiles):
                                    if g[0] == "qk":
                                        _, ft, qc = g
                                        # copy out w/ per-part bias + Q scale
                                        nc.vector.tensor_scalar(
                                            out=qkT_sb[
                                                :, ft, qc * 512 : (qc + 1) * 512
                                            ],
                                            in0=t,
                                            scalar1=qkb_sb[:, ft : ft + 1],
                                            scalar2=(
                                                sQ if ft < FT_QK // 2 else 1.0
                                            ),
                                            op0=mybir.AluOpType.add,
                                            op1=mybir.AluOpType.mult,
                                        )
                                    else:
                                        _, st, _ = g
                                        nc.vector.tensor_tensor(
                                            out=v_sb[:, st, :, 0:HD],
                                            in0=t.rearrange(
                                                "p (h d) -> p h d", h=H
                                            ),
                                            in1=vb_sb.rearrange(
                                                "p (h d) -> p h d", h=H
                                            ),
                                            op=mybir.AluOpType.add,
                                        )

                        g_pair0 = [("qk", ft, qc) for ft in (0, 4)
                                   for qc in range(QC)]
                        g_rest = [("qk", ft, qc) for ft in (1, 5, 2, 6, 3, 7)
                                  for qc in range(QC)]
                        g_rest += [("v", st, 0) for st in range(NT)]

                        emit_blocks(g_pair0)
                        expT0 = new_expT()
                        es0 = emit_scores_exps(0)
                        emit_blocks(g_rest)

                    # ---- phase 2/3: head-pair pipeline ----
                    with tc.tile_pool(name="ps_acc", bufs=2,
                                      space="PSUM") as ps_acc:
                        emit_mults(es0, expT0)
                        expT = {0: expT0}
                        for hp in range(H // 2):
                            nxt = hp + 1
                            es_n = None
                            if nxt < H // 2:
                                es_n = emit_scores_exps(nxt)
                                expT[nxt] = new_expT()
                            if hp < H // 2 - 1:
                                for qc in range(QC):
                                    emit_pv_norm(hp, expT[hp], qc, ps_acc)
                                if es_n is not None:
                                    emit_mults(es_n, expT[nxt])
                            else:
                                for qc in range(QC):
                                    emit_pv_norm(hp, expT[hp], qc, ps_acc)
                                    emit_ph3(qc, ps_acc)
    nc.finalize()
    return nc


def kernel(x, graph_bias, in_proj_w, in_proj_b, out_proj_w, out_proj_b,
           bias_strength):
    x = np.asarray(x, dtype=np.float32)
    graph_bias = np.asarray(graph_bias, dtype=np.float32)
    in_proj_w = np.asarray(in_proj_w, dtype=np.float32)
    in_proj_b = np.asarray(in_proj_b, dtype=np.float32)
    out_proj_w = np.asarray(out_proj_w, dtype=np.float32)
    out_proj_b = np.asarray(out_proj_b, dtype=np.float32)
    alpha = 1.0 / (1.0 + np.exp(-float(np.asarray(bias_strength))))
    ten_a = 10.0 * alpha

    key = round(ten_a, 9)
    if key not in _CACHE:
        _CACHE[key] = build_nc(ten_a)
    nc = _CACHE[key]

    wT = in_proj_w.T                                # [512, 1536]
    woT = np.ascontiguousarray(out_proj_w.T).astype(np.float16)
    qkb = np.ascontiguousarray(
        in_proj_b[: 2 * D].reshape(FT_QK, P).T      # [128, 8]
    )
    vb = in_proj_b[2 * D :].astype(np.float16)
    bo = out_proj_b.astype(np.float16)
    gb16 = graph_bias.astype(np.float16)

    in_maps = []
    for b in range(B):
        xwb = np.concatenate([x[b].T, wT], axis=1).astype(np.float16)
        in_maps.append({
            "xw": np.ascontiguousarray(xwb),
            "gb": np.ascontiguousarray(gb16[b]),
            "woT": woT,
            "qkb": qkb,
            "vb": vb,
            "bo": bo.reshape(1, D),
            "ones1": np.ones((1, P), dtype=np.float16),
        })

    global _saved_in_maps
    _saved_in_maps = in_maps
    res = run_bass_kernel_spmd(nc, in_maps, core_ids=list(range(B)))
    out = np.stack([res.results[b]["y"] for b in range(B)], axis=0)
    return out.astype(np.float32)
